# revision 36
# baseline (speedup 1.0000x reference)
"""Transformer decoder layer (masked self-attn + cross-attn + FFN, 3x LayerNorm)
for Trainium2, data-parallel over batch across 8 NeuronCores.

Per-core problem: L=1024 tokens, D=1024 model dim, H=16 heads x 64, DFF=4096.

v2 design (prior baseline measured 1.79 ms/core on the NTFF profile):
- Host prepacks everything to bf16 in SBUF-ready layouts; the kernel DMAs
  weights/activations straight into place (no on-device fp32 staging/casts).
- Blocked DMA_TRANSPOSE: one instruction per [1024,1024] matrix (DRAM source)
  or per [128,1024] row-block (SBUF source) instead of per-128x128 tile.
- V is projected directly into [key-token, h*65] layout (stationary = x^T
  chunk, moving = wv) with a ones column per head, so the O^T matmul also
  emits the softmax denominator; V needs no transposes at all.
- S^T = K^T.T @ Q^T per head, two heads packed into the PE via row groups.
- exp on ScalarE over 2-bank PSUM groups; per-phase batched Rsqrt for the
  LayerNorms (one ACT instruction per LN phase -> no table-set thrash).
- FFN uses h1 chunks as the stationary operand so y lands untransposed.
- SBUF: one shared pool of 10 x ~16.6KB slots (tag "m") recycled across
  phases + small pools; PSUM: 2x[128,2,512] "sc" + 2x[128,2,512] "pj".
"""

import sys

sys.path.insert(0, "/opt/trn_rl_repo")

import numpy as np
import ml_dtypes

import concourse.bass as bass
import concourse.mybir as mybir
import concourse.tile as tile
from concourse import bacc
from concourse.bass_utils import run_bass_kernel_spmd

FP32 = mybir.dt.float32
BF16 = mybir.dt.bfloat16
AF = mybir.ActivationFunctionType
ALU = mybir.AluOpType

B = 8
L = 1024
D = 1024
H = 16
DK = 64
DFF = 4096
P = 128
NT = L // P  # 8 l-tiles
DT = D // P  # 8 d-tiles
NP = H // 2  # 8 head pairs
LC = 512
NLC = L // LC  # 2
FQ = 4  # ffn dff quarters
FT = DFF // FQ // P  # 8 f-tiles per quarter
EPS = 1e-5

NP_BF16 = ml_dtypes.bfloat16

INPUT_SPECS = {
    "x": ([L, D], BF16),
    "enc": ([L, D], BF16),
    "wq_m": ([D, D], BF16), "wk_m": ([D, D], BF16), "wv_m": ([D, D], BF16),
    "wq_c": ([D, D], BF16), "wk_c": ([D, D], BF16), "wv_c": ([D, D], BF16),
    "bqk_m": ([2, D], FP32), "bqk_c": ([2, D], FP32),
    "bv_m": ([D], BF16), "bv_c": ([D], BF16),
    "w1": ([D, DFF], BF16), "w2": ([DFF, D], BF16),
    "b1": ([DFF], FP32), "b2": ([D], BF16),
    "g1": ([D], BF16), "bb1": ([D], BF16),
    "g2": ([D], BF16), "bb2": ([D], BF16),
}


def _bcast_ap(ap, parts=P):
    """Broadcast a 1-D DRAM AP across `parts` partitions (step-0 partition dim)."""
    return bass.AP(tensor=ap.tensor, offset=ap.offset, ap=[[0, parts]] + list(ap.ap))


def build(stop_after=None):
    nc = bacc.Bacc("TRN2", target_bir_lowering=False, debug=False, num_devices=B)

    dram = {}
    for name, (shape, dt) in INPUT_SPECS.items():
        dram[name] = nc.dram_tensor(name, shape, dt, kind="ExternalInput")
    out_d = nc.dram_tensor("out", [L, D], FP32, kind="ExternalOutput")

    with tile.TileContext(nc) as tc:
        _emit(nc, tc, dram, out_d, stop_after)
    nc.compile()
    return nc


def _emit(nc, tc, dram, out_d, stop_after):
    with tc.tile_pool(name="const", bufs=1) as const, \
         tc.tile_pool(name="m", bufs=10) as m, \
         tc.tile_pool(name="heads", bufs=3) as heads, \
         tc.tile_pool(name="stage", bufs=2, side="left") as stage, \
         tc.tile_pool(name="pj", bufs=2, space=bass.MemorySpace.PSUM) as psum_pj, \
         tc.tile_pool(name="sc", bufs=2, space=bass.MemorySpace.PSUM) as psum_sc:
        _body(nc, dram, out_d, stop_after, const, m, heads, stage, psum_pj, psum_sc)


def _body(nc, dram, out_d, stop_after, const, m, heads, stage, psum_pj, psum_sc):
    def ap(name):
        return dram[name].ap()

    # ---- constants ----
    eps_t = const.tile([P, 1], FP32)
    nc.vector.memset(eps_t, EPS)

    # causal 0/1 masks for diagonal blocks: mask[i][kk, qq] = 1 if qq >= kk + i*128
    mask_bf = const.tile([P, 4, LC], BF16)
    for i in range(4):
        m32 = stage.tile([P, LC], FP32, tag="zb")
        nc.vector.memset(m32, 1.0)
        nc.gpsimd.affine_select(
            out=m32,
            in_=m32,
            compare_op=ALU.is_ge,
            fill=0.0,
            base=-(i * P),
            pattern=[[1, LC]],
            channel_multiplier=-1,
        )
        nc.vector.tensor_copy(mask_bf[:, i, :], m32)

    # ---- inputs (xT first: the first projection waits on it) ----
    xT = m.tile([P, DT, L], BF16, tag="m")
    for lh in range(2):
        nc.sync.dma_start(
            xT[:, :, lh * LC:(lh + 1) * LC],
            ap("x")[lh * LC:(lh + 1) * LC, :],
            transpose=True,
        )
    encT = m.tile([P, DT, L], BF16, tag="m")
    nc.sync.dma_start(encT, ap("enc"), transpose=True)

    # ---- helpers ----
    # Weight/param loads go through SWDGE (gpsimd): HWDGE rings serialize
    # against xbar-transpose mode switches, so plain loads there stall behind
    # every DMA_TRANSPOSE in flight. SWDGE rings do not.
    def load_w(name):
        w = m.tile([P, DT, D], BF16, tag="m")
        nc.gpsimd.dma_start(w, ap(name).rearrange("(dt p) c -> p dt c", p=P))
        return w

    def project_qk(wname, b_col, j, srcT, lc_outer=False):
        # returns [128(i*64+k), NP, L]: per head-pair column block of W^T srcT + b
        # lc_outer: emit all head-pairs for l-chunk 0 first so consumers of the
        # first chunk (and producers of only the first srcT l-columns) pipeline.
        dst = m.tile([P, NP, L], BF16, tag="m")
        w = load_w(wname)
        if lc_outer:
            for lc in range(NLC):
                for pr in range(NP):
                    ps = psum_pj.tile([P, 1, LC], FP32, tag="pj")
                    for dt in range(DT):
                        nc.tensor.matmul(
                            ps[:, 0, :],
                            w[:, dt, pr * P:(pr + 1) * P],
                            srcT[:, dt, lc * LC:(lc + 1) * LC],
                            start=(dt == 0),
                            stop=(dt == DT - 1),
                        )
                    nc.vector.tensor_scalar_add(
                        dst[:, pr, lc * LC:(lc + 1) * LC],
                        ps[:, 0, :],
                        b_col[:, j, pr:pr + 1],
                    )
            return dst
        for pr in range(NP):
            ps = psum_pj.tile([P, NLC, LC], FP32, tag="pj")
            for dt in range(DT):
                lhsT = w[:, dt, pr * P:(pr + 1) * P]
                for lc in range(NLC):
                    nc.tensor.matmul(
                        ps[:, lc, :],
                        lhsT,
                        srcT[:, dt, lc * LC:(lc + 1) * LC],
                        start=(dt == 0),
                        stop=(dt == DT - 1),
                    )
            nc.vector.tensor_scalar_add(
                dst[:, pr, :].rearrange("p (a b) -> p a b", a=NLC),
                ps,
                b_col[:, j, pr:pr + 1],
            )
        return dst

    def project_v(wname, bv_bc, srcT):
        # V [128(lk), NT, H*65]: V[:, kt, h*65+v] = (srcT_chunk.T @ wv)[lk, h*64+v] + bv
        # col 65*h+64 is a ones column.
        V = m.tile([P, NT, H * 65], BF16, tag="m")
        w = load_w(wname)
        for kt in range(NT):
            ps = psum_pj.tile([P, NLC, LC], FP32, tag="pj")
            for dt in range(DT):
                lhsT = srcT[:, dt, kt * P:(kt + 1) * P]
                for lc in range(NLC):
                    nc.tensor.matmul(
                        ps[:, lc, :],
                        lhsT,
                        w[:, dt, lc * LC:(lc + 1) * LC],
                        start=(dt == 0),
                        stop=(dt == DT - 1),
                    )
            Vv = V[:, kt, :].rearrange("p (h c) -> p h c", c=65)
            for lc in range(NLC):
                nc.vector.tensor_add(
                    Vv[:, lc * 8:(lc + 1) * 8, 0:64],
                    ps[:, lc, :].rearrange("p (h c) -> p h c", c=64),
                    bv_bc[:, lc * LC:(lc + 1) * LC].rearrange(
                        "p (h c) -> p h c", c=64
                    ),
                )
        nc.vector.memset(
            V.rearrange("p a (h c) -> p a h c", c=65)[:, :, :, 64:65], 1.0
        )
        return V

    def attention(out_sa, causal, qt, kt, V):
        for pr in range(NP):
            eS0 = m.tile([P, NT, L], BF16, tag="m")
            eS1 = m.tile([P, NT, L], BF16, tag="m")
            eS = [eS0, eS1]
            for lc in range(NLC):
                kts = list(range(4)) if (causal and lc == 0) else list(range(NT))
                for g0 in range(0, len(kts), 2):
                    grp = kts[g0:g0 + 2]
                    ps0 = psum_sc.tile([P, 2, LC], FP32, tag="sc")
                    ps1 = psum_sc.tile([P, 2, LC], FP32, tag="sc")
                    pss = [ps0, ps1]
                    for j, kt_ in enumerate(grp):
                        for i in range(2):
                            r0 = i * 64
                            nc.tensor.matmul(
                                pss[i][:, j, :],
                                kt[r0:r0 + 64, pr, kt_ * P:(kt_ + 1) * P],
                                qt[r0:r0 + 64, pr, lc * LC:(lc + 1) * LC],
                                start=True,
                                stop=True,
                                tile_position=(r0, 0),
                            )
                    for i in range(2):
                        nc.scalar.activation(
                            eS[i][:, grp[0]:grp[0] + len(grp),
                                  lc * LC:(lc + 1) * LC],
                            pss[i][:, 0:len(grp), :],
                            AF.Exp,
                            scale=0.125,
                        )
                    if causal:
                        for kt_ in grp:
                            if kt_ >= 4 * lc:
                                mi = kt_ - 4 * lc
                                for i in range(2):
                                    nc.vector.tensor_mul(
                                        eS[i][:, kt_, lc * LC:(lc + 1) * LC],
                                        eS[i][:, kt_, lc * LC:(lc + 1) * LC],
                                        mask_bf[:, mi, :],
                                    )
            # O^T rows 0:64 + softmax denominator row 64 (ones column of V)
            for i in range(2):
                h = 2 * pr + i
                av = psum_pj.tile([P, NLC, LC], FP32, tag="pj")
                for lc in range(NLC):
                    kts = list(range(4)) if (causal and lc == 0) else list(range(NT))
                    for j, kt_ in enumerate(kts):
                        nc.tensor.matmul(
                            av[0:65, lc, :],
                            V[:, kt_, h * 65:h * 65 + 65],
                            eS[i][:, kt_, lc * LC:(lc + 1) * LC],
                            start=(j == 0),
                            stop=(j == len(kts) - 1),
                        )
                ot = heads.tile([80, L], BF16, tag="ot")
                nc.vector.memset(ot[64:80, :], 0.0)
                otv = ot[0:65, :].rearrange("p (a b) -> p a b", a=NLC)
                if i == 0:
                    nc.vector.tensor_copy(otv, av[0:65])
                else:
                    # balance PSUM evacuations across DVE and ScalarE
                    nc.scalar.activation(otv, av[0:65], AF.Identity)
                otr = heads.tile([P, NT, 80], BF16, tag="otr")
                nc.sync.dma_start(otr, ot, transpose=True)
                rcp = heads.tile([P, NT, 1], FP32, tag="rcp")
                nc.vector.reciprocal(rcp, otr[:, :, 64:65])
                nc.vector.tensor_mul(
                    out_sa.rearrange("p lt (hh c) -> p lt hh c", c=64)[:, :, h, :],
                    otr[:, :, 0:64],
                    rcp.broadcast_to([P, NT, 64]),
                )

    def ln_stats_block(res_lt, sums_lt, ssq_lt):
        # Sigma r and Sigma r^2 both on ScalarE (parallel to the DVE chain)
        dump = stage.tile([P, D], FP32, tag="zf")
        nc.scalar.activation(dump, res_lt, AF.Identity, accum_out=sums_lt)
        dump2 = stage.tile([P, D], FP32, tag="zf")
        nc.scalar.activation(dump2, res_lt, AF.Square, accum_out=ssq_lt)

    def ln_half_scalars(sums, ssq, rsq, mrs, sl):
        # mean = sums/D; var = ssq/D - mean^2; rstd = 1/sqrt(var+eps)
        mh = stage.tile([P, 4, 1], FP32, tag="mh")
        nc.vector.tensor_scalar_mul(mh, sums[:, sl, :], 1.0 / D)
        m2 = stage.tile([P, 4, 1], FP32, tag="m2")
        nc.vector.tensor_mul(m2, mh, mh)
        v1 = stage.tile([P, 4, 1], FP32, tag="v1")
        nc.vector.scalar_tensor_tensor(
            v1, ssq[:, sl, :], 1.0 / D, m2, op0=ALU.mult, op1=ALU.subtract
        )
        sq = stage.tile([P, 4, 1], FP32, tag="sq")
        nc.scalar.activation(sq, v1, AF.Sqrt, bias=eps_t[:, 0:1])
        nc.vector.reciprocal(rsq[:, sl, :], sq)
        nc.vector.tensor_mul(mrs[:, sl, :], mh, rsq[:, sl, :])

    def ln_phase(a_big, b_big, g_t, b_t, emit_block, res_name="res"):
        # residual r = a+b with free-dim sum accumulated in the same DVE op;
        # stats batched per half (4 blocks) so downstream work starts early.
        res = m.tile([P, NT, D], BF16, tag="m")
        sums = stage.tile([P, NT, 1], FP32, tag="sums")
        ssq = stage.tile([P, NT, 1], FP32, tag="ssq")
        rsq = stage.tile([P, NT, 1], FP32, tag="rsq")
        mrs = stage.tile([P, NT, 1], FP32, tag="mrs")
        for hf in range(2):
            lts = range(hf * 4, hf * 4 + 4)
            for lt in lts:
                nc.vector.tensor_add(res[:, lt, :], a_big[:, lt, :], b_big[:, lt, :])
                ln_stats_block(res[:, lt, :], sums[:, lt, :], ssq[:, lt, :])
            sl = slice(hf * 4, hf * 4 + 4)
            ln_half_scalars(sums, ssq, rsq, mrs, sl)
            for lt in lts:
                emit_block(lt, res, rsq, mrs)
        return res

    def ln_finish(dst, res_lt, rsq_lt, mrs_lt, g_t, b_t, via=None):
        z = via if via is not None else dst
        nc.vector.tensor_scalar(
            z, res_lt, rsq_lt, mrs_lt, op0=ALU.mult, op1=ALU.subtract
        )
        nc.vector.tensor_mul(dst, z, g_t)
        nc.vector.tensor_add(dst, dst, b_t)

    def tap(src_big):
        for lt in range(NT):
            o = stage.tile([P, D], FP32, tag="zf")
            nc.vector.tensor_copy(o, src_big[:, lt, :])
            nc.sync.dma_start(out_d.ap()[lt * P:(lt + 1) * P, :], o)

    # consts needed by the self-attention projections (kept ahead of the
    # bulk const DMAs so the wq load isn't far behind them on the SWDGE ring)
    bqk = {}
    t_bqk_m = const.tile([P, 2, NP], FP32, tag="bqk_m")
    nc.gpsimd.dma_start(t_bqk_m, ap("bqk_m").rearrange("j (pr p) -> p j pr", p=P))
    bqk["bqk_m"] = t_bqk_m
    bcast = {}
    t_bv_m = const.tile([P, D], BF16, tag="bc_bv_m")
    nc.gpsimd.dma_start(t_bv_m, _bcast_ap(ap("bv_m")))
    bcast["bv_m"] = t_bv_m

    # ================= self attention =================
    qt_s = project_qk("wq_m", bqk["bqk_m"], 0, xT)
    kt_s = project_qk("wk_m", bqk["bqk_m"], 1, xT)
    V_s = project_v("wv_m", bcast["bv_m"], xT)
    # xT's slot is recycled after V_s projection (last reader)

    sa = m.tile([P, NT, D], BF16, tag="m")
    attention(sa, True, qt_s, kt_s, V_s)
    if stop_after == "sa":
        tap(sa)
        return

    # remaining consts (first used at/after the cross projections)
    t_bqk_c = const.tile([P, 2, NP], FP32, tag="bqk_c")
    nc.gpsimd.dma_start(t_bqk_c, ap("bqk_c").rearrange("j (pr p) -> p j pr", p=P))
    bqk["bqk_c"] = t_bqk_c
    b1_col = const.tile([P, DFF // P], FP32)
    nc.gpsimd.dma_start(b1_col, ap("b1").rearrange("(ft p) -> p ft", p=P))
    for name in ("bv_c", "b2", "g1", "bb1", "g2", "bb2"):
        t = const.tile([P, D], BF16, tag=f"bc_{name}")
        nc.gpsimd.dma_start(t, _bcast_ap(ap(name)))
        bcast[name] = t

    # cross K/V projections (can fill PE gaps at the tail of self-attn)
    x_res = m.tile([P, NT, D], BF16, tag="m")
    nc.gpsimd.dma_start(x_res, ap("x").rearrange("(lt p) d -> p lt d", p=P))
    kt_c = project_qk("wk_c", bqk["bqk_c"], 1, encT)
    V_c = project_v("wv_c", bcast["bv_c"], encT)

    # ---- residual + LN1 -> x1 (bf16) and x1T ----
    x1 = m.tile([P, NT, D], BF16, tag="m")
    x1T = m.tile([P, DT, L], BF16, tag="m")

    def emit_ln1(lt, res, rsq, mrs):
        z = stage.tile([P, D], BF16, tag="zb")
        ln_finish(x1[:, lt, :], res[:, lt, :], rsq[:, lt, :], mrs[:, lt, :],
                  bcast["g1"], bcast["bb1"], via=z)
        nc.sync.dma_start(
            x1T[:, :, lt * P:(lt + 1) * P], x1[:, lt, :], transpose=True
        )

    ln_phase(x_res, sa, bcast["g1"], bcast["bb1"], emit_ln1)
    if stop_after == "x1":
        tap(x1)
        return

    # ================= cross attention =================
    qt_c = project_qk("wq_c", bqk["bqk_c"], 0, x1T, lc_outer=True)
    ca = m.tile([P, NT, D], BF16, tag="m")
    attention(ca, False, qt_c, kt_c, V_c)
    if stop_after == "ca":
        tap(ca)
        return

    # ---- residual + LN2 -> x2 (bf16) and x2T ----
    x2 = m.tile([P, NT, D], BF16, tag="m")
    x2T = m.tile([P, DT, L], BF16, tag="m")

    def emit_ln2(lt, res, rsq, mrs):
        z = stage.tile([P, D], BF16, tag="zb")
        ln_finish(x2[:, lt, :], res[:, lt, :], rsq[:, lt, :], mrs[:, lt, :],
                  bcast["g2"], bcast["bb2"], via=z)
        nc.sync.dma_start(
            x2T[:, :, lt * P:(lt + 1) * P], x2[:, lt, :], transpose=True
        )

    ln_phase(x1, ca, bcast["g2"], bcast["bb2"], emit_ln2)
    if stop_after == "x2":
        tap(x2)
        return

    # ================= FFN (dff quarters) =================
    y_bf = m.tile([P, NT, D], BF16, tag="m")
    res3 = None
    sums3 = stage.tile([P, NT, 1], FP32, tag="sums")
    ssq3 = stage.tile([P, NT, 1], FP32, tag="ssq")
    for q in range(FQ):
        w1 = m.tile([P, DT, FT * P], BF16, tag="m")
        nc.gpsimd.dma_start(
            w1,
            ap("w1")[:, q * FT * P:(q + 1) * FT * P].rearrange(
                "(dt p) c -> p dt c", p=P
            ),
        )
        h1 = m.tile([P, FT, L], BF16, tag="m")
        if q == 0:
            # lc-outer: h1 for the first l-half only needs x2T's first 512
            # l-columns (LN2 blocks 0..3) -> FFN starts during LN2.
            for lc in range(NLC):
                for ft in range(FT):
                    ps = psum_sc.tile([P, 1, LC], FP32, tag="sc")
                    for dt in range(DT):
                        nc.tensor.matmul(
                            ps[:, 0, :],
                            w1[:, dt, ft * P:(ft + 1) * P],
                            x2T[:, dt, lc * LC:(lc + 1) * LC],
                            start=(dt == 0),
                            stop=(dt == DT - 1),
                        )
                    nc.scalar.activation(
                        h1[:, ft, lc * LC:(lc + 1) * LC],
                        ps[:, 0, :],
                        AF.Relu,
                        bias=b1_col[:, q * FT + ft:q * FT + ft + 1],
                    )
        else:
            for ft in range(FT):
                ps = psum_sc.tile([P, NLC, LC], FP32, tag="sc")
                for dt in range(DT):
                    lhsT = w1[:, dt, ft * P:(ft + 1) * P]
                    for lc in range(NLC):
                        nc.tensor.matmul(
                            ps[:, lc, :],
                            lhsT,
                            x2T[:, dt, lc * LC:(lc + 1) * LC],
                            start=(dt == 0),
                            stop=(dt == DT - 1),
                        )
                nc.scalar.activation(
                    h1[:, ft, :].rearrange("p (a b) -> p a b", a=NLC),
                    ps,
                    AF.Relu,
                    bias=b1_col[:, q * FT + ft:q * FT + ft + 1],
                )
        w2 = m.tile([P, FT, D], BF16, tag="m")
        nc.gpsimd.dma_start(
            w2,
            ap("w2")[q * FT * P:(q + 1) * FT * P, :].rearrange(
                "(ft p) c -> p ft c", p=P
            ),
        )
        if q == FQ - 1:
            res3 = m.tile([P, NT, D], BF16, tag="m")
            # pre-residual x2 + y(q0..q2): runs on DVE during FFN1 of the
            # last quarter, so the q3 evacuation is a single op per block
            pre3 = m.tile([P, NT, D], BF16, tag="m")
            for lb in range(NT):
                nc.vector.tensor_add(pre3[:, lb, :], y_bf[:, lb, :], x2[:, lb, :])
        for lb in range(NT):
            ps = psum_pj.tile([P, NLC, LC], FP32, tag="pj")
            for ft in range(FT):
                lhsT = h1[:, ft, lb * P:(lb + 1) * P]
                for lc in range(NLC):
                    nc.tensor.matmul(
                        ps[:, lc, :],
                        lhsT,
                        w2[:, ft, lc * LC:(lc + 1) * LC],
                        start=(ft == 0),
                        stop=(ft == FT - 1),
                    )
            psv = ps.rearrange("p a b -> p (a b)")
            if q == 0:
                nc.vector.tensor_add(y_bf[:, lb, :], psv, bcast["b2"])
            elif q < FQ - 1:
                nc.vector.tensor_add(y_bf[:, lb, :], y_bf[:, lb, :], psv)
            else:
                nc.vector.scalar_tensor_tensor(
                    res3[:, lb, :], psv, 1.0, pre3[:, lb, :],
                    op0=ALU.mult, op1=ALU.add, accum_out=sums3[:, lb, :],
                )
                dump2 = stage.tile([P, D], FP32, tag="zf")
                nc.scalar.activation(
                    dump2, res3[:, lb, :], AF.Square, accum_out=ssq3[:, lb, :]
                )

    # ---- final LN (reuses ln2 params), fp32 out, per 4-block half ----
    rsq = stage.tile([P, NT, 1], FP32, tag="rsq")
    mrs = stage.tile([P, NT, 1], FP32, tag="mrs")
    for hf in range(2):
        sl = slice(hf * 4, hf * 4 + 4)
        ln_half_scalars(sums3, ssq3, rsq, mrs, sl)
        for lt in range(hf * 4, hf * 4 + 4):
            # normalize in bf16 (2x/4x DVE modes); SWDGE casts bf16->fp32
            # during the store
            o = stage.tile([P, D], BF16, tag="zb")
            z = stage.tile([P, D], BF16, tag="zb")
            ln_finish(o, res3[:, lt, :], rsq[:, lt, :], mrs[:, lt, :],
                      bcast["g2"], bcast["bb2"], via=z)
            nc.gpsimd.dma_start(out_d.ap()[lt * P:(lt + 1) * P, :], o)


_NC_CACHE = {}


def _get_nc(stop_after=None):
    key = stop_after
    if key not in _NC_CACHE:
        _NC_CACHE[key] = build(stop_after)
    return _NC_CACHE[key]


def _pack_weights(inputs):
    """Host-side prepack: cast to bf16 and lay out as the kernel expects."""
    f32 = lambda k: np.ascontiguousarray(np.asarray(inputs[k], dtype=np.float32))
    bf = lambda a: np.ascontiguousarray(np.asarray(a, dtype=NP_BF16))

    def attn_w(k):
        # [H, D, DK] -> [D, H*DK] bf16
        w = f32(k).transpose(1, 0, 2).reshape(D, H * DK)
        return bf(w)

    return {
        "wq_m": attn_w("m_wq"), "wk_m": attn_w("m_wk"), "wv_m": attn_w("m_wv"),
        "wq_c": attn_w("c_wq"), "wk_c": attn_w("c_wk"), "wv_c": attn_w("c_wv"),
        "bqk_m": np.ascontiguousarray(
            np.stack([f32("m_bq").reshape(-1), f32("m_bk").reshape(-1)])
        ),
        "bqk_c": np.ascontiguousarray(
            np.stack([f32("c_bq").reshape(-1), f32("c_bk").reshape(-1)])
        ),
        "bv_m": bf(f32("m_bv").reshape(-1)),
        "bv_c": bf(f32("c_bv").reshape(-1)),
        "w1": bf(f32("ff_w1")),
        "w2": bf(f32("ff_w2")),
        "b1": f32("ff_b1"),
        "b2": bf(f32("ff_b2")),
        "g1": bf(f32("ln1_g")), "bb1": bf(f32("ln1_b")),
        "g2": bf(f32("ln2_g")), "bb2": bf(f32("ln2_b")),
    }


def _make_in_maps(inputs):
    xs = np.ascontiguousarray(
        np.asarray(inputs["decoder_embedding"], dtype=np.float32).astype(NP_BF16)
    )
    es = np.ascontiguousarray(
        np.asarray(inputs["encoder_output"], dtype=np.float32).astype(NP_BF16)
    )
    packed = _pack_weights(inputs)
    return [{**packed, "x": xs[b], "enc": es[b]} for b in range(B)]


def _gather(res):
    return np.stack([res.results[b]["out"] for b in range(B)], axis=0).astype(np.float32)


def kernel(**inputs):
    nc = _get_nc()
    res = run_bass_kernel_spmd(nc, _make_in_maps(inputs), core_ids=list(range(B)))
    return _gather(res)


# revision 51
# speedup vs baseline: 1.0687x; 1.0687x over previous
"""Transformer decoder layer (masked self-attn + cross-attn + FFN, 3x LayerNorm)
for Trainium2, data-parallel over batch across 8 NeuronCores.

Per-core problem: L=1024 tokens, D=1024 model dim, H=16 heads x 64, DFF=4096.

v2 design (prior baseline measured 1.79 ms/core on the NTFF profile):
- Host prepacks everything to bf16 in SBUF-ready layouts; the kernel DMAs
  weights/activations straight into place (no on-device fp32 staging/casts).
- Blocked DMA_TRANSPOSE: one instruction per [1024,1024] matrix (DRAM source)
  or per [128,1024] row-block (SBUF source) instead of per-128x128 tile.
- V is projected directly into [key-token, h*65] layout (stationary = x^T
  chunk, moving = wv) with a ones column per head, so the O^T matmul also
  emits the softmax denominator; V needs no transposes at all.
- S^T = K^T.T @ Q^T per head, two heads packed into the PE via row groups.
- exp on ScalarE over 2-bank PSUM groups; per-phase batched Rsqrt for the
  LayerNorms (one ACT instruction per LN phase -> no table-set thrash).
- FFN uses h1 chunks as the stationary operand so y lands untransposed.
- SBUF: one shared pool of 10 x ~16.6KB slots (tag "m") recycled across
  phases + small pools; PSUM: 2x[128,2,512] "sc" + 2x[128,2,512] "pj".
"""

import sys

sys.path.insert(0, "/opt/trn_rl_repo")

import numpy as np
import ml_dtypes

import concourse.bass as bass
import concourse.mybir as mybir
import concourse.tile as tile
from concourse import bacc
from concourse.bass_utils import run_bass_kernel_spmd

FP32 = mybir.dt.float32
BF16 = mybir.dt.bfloat16
AF = mybir.ActivationFunctionType
ALU = mybir.AluOpType

B = 8
L = 1024
D = 1024
H = 16
DK = 64
DFF = 4096
P = 128
NT = L // P  # 8 l-tiles
DT = D // P  # 8 d-tiles
NP = H // 2  # 8 head pairs
LC = 512
NLC = L // LC  # 2
FQ = 4  # ffn dff quarters
FT = DFF // FQ // P  # 8 f-tiles per quarter
EPS = 1e-5

NP_BF16 = ml_dtypes.bfloat16

INPUT_SPECS = {
    "x": ([L, D], BF16),
    "mask": ([4, P, LC], BF16),
    "enc": ([L, D], BF16),
    "wq_m": ([D, D], BF16), "wk_m": ([D, D], BF16), "wv_m": ([D, D], BF16),
    "wq_c": ([D, D], BF16), "wk_c": ([D, D], BF16), "wv_c": ([D, D], BF16),
    "bqk_m": ([2, D], FP32), "bqk_c": ([2, D], FP32),
    "bv_m": ([D], BF16), "bv_c": ([D], BF16),
    "w1": ([D, DFF], BF16), "w2": ([DFF, D], BF16),
    "b1": ([DFF], FP32), "b2": ([D], BF16),
    "g1": ([D], BF16), "bb1": ([D], BF16),
    "g2": ([D], BF16), "bb2": ([D], BF16),
}


def _bcast_ap(ap, parts=P):
    """Broadcast a 1-D DRAM AP across `parts` partitions (step-0 partition dim)."""
    return bass.AP(tensor=ap.tensor, offset=ap.offset, ap=[[0, parts]] + list(ap.ap))


def build(stop_after=None):
    nc = bacc.Bacc("TRN2", target_bir_lowering=False, debug=False, num_devices=B)

    dram = {}
    for name, (shape, dt) in INPUT_SPECS.items():
        dram[name] = nc.dram_tensor(name, shape, dt, kind="ExternalInput")
    out_d = nc.dram_tensor("out", [L, D], FP32, kind="ExternalOutput")

    with tile.TileContext(nc) as tc:
        _emit(nc, tc, dram, out_d, stop_after)
    nc.compile()
    return nc


def _emit(nc, tc, dram, out_d, stop_after):
    with tc.tile_pool(name="const", bufs=1) as const, \
         tc.tile_pool(name="m", bufs=10) as m, \
         tc.tile_pool(name="heads", bufs=3) as heads, \
         tc.tile_pool(name="stage", bufs=2, side="left") as stage, \
         tc.tile_pool(name="pj", bufs=2, space=bass.MemorySpace.PSUM) as psum_pj, \
         tc.tile_pool(name="sc", bufs=2, space=bass.MemorySpace.PSUM) as psum_sc:
        _body(nc, dram, out_d, stop_after, const, m, heads, stage, psum_pj, psum_sc)


def _body(nc, dram, out_d, stop_after, const, m, heads, stage, psum_pj, psum_sc):
    def ap(name):
        return dram[name].ap()

    # ---- constants ----
    eps_t = const.tile([P, 1], FP32)
    nc.vector.memset(eps_t, EPS)

    # causal 0/1 masks for diagonal blocks (host-built constant):
    # mask[i][kk, qq] = 1 if qq >= kk + i*128
    mask_bf = const.tile([P, 4, LC], BF16)

    # ---- inputs (xT half 0 first: the first projection chunk waits on it;
    # half 1 and encT are emitted later, ordered by first need) ----
    xT = m.tile([P, DT, L], BF16, tag="m")
    nc.sync.dma_start(
        xT[:, :, 0:LC], ap("x")[0:LC, :], transpose=True,
    )

    # ---- helpers ----
    # Weight/param loads go through SWDGE (gpsimd): HWDGE rings serialize
    # against xbar-transpose mode switches, so plain loads there stall behind
    # every DMA_TRANSPOSE in flight. SWDGE rings do not.
    def load_w(name):
        w = m.tile([P, DT, D], BF16, tag="m")
        nc.gpsimd.dma_start(w, ap(name).rearrange("(dt p) c -> p dt c", p=P))
        return w

    def project_qk(wname, b_col, j, srcT, lc_outer=False, w_pre=None, between_lc=None):
        # returns [128(i*64+k), NP, L]: per head-pair column block of W^T srcT + b
        # lc_outer: emit all head-pairs for l-chunk 0 first so consumers of the
        # first chunk (and producers of only the first srcT l-columns) pipeline.
        dst = m.tile([P, NP, L], BF16, tag="m")
        w = w_pre if w_pre is not None else load_w(wname)
        if lc_outer:
            for lc in range(NLC):
                if lc == 1 and between_lc is not None:
                    between_lc()
                for pr in range(NP):
                    ps = psum_pj.tile([P, 1, LC], FP32, tag="pj")
                    for dt in range(DT):
                        nc.tensor.matmul(
                            ps[:, 0, :],
                            w[:, dt, pr * P:(pr + 1) * P],
                            srcT[:, dt, lc * LC:(lc + 1) * LC],
                            start=(dt == 0),
                            stop=(dt == DT - 1),
                        )
                    nc.vector.tensor_scalar_add(
                        dst[:, pr, lc * LC:(lc + 1) * LC],
                        ps[:, 0, :],
                        b_col[:, j, pr:pr + 1],
                    )
            return dst
        for pr in range(NP):
            ps = psum_pj.tile([P, NLC, LC], FP32, tag="pj")
            for dt in range(DT):
                lhsT = w[:, dt, pr * P:(pr + 1) * P]
                for lc in range(NLC):
                    nc.tensor.matmul(
                        ps[:, lc, :],
                        lhsT,
                        srcT[:, dt, lc * LC:(lc + 1) * LC],
                        start=(dt == 0),
                        stop=(dt == DT - 1),
                    )
            nc.vector.tensor_scalar_add(
                dst[:, pr, :].rearrange("p (a b) -> p a b", a=NLC),
                ps,
                b_col[:, j, pr:pr + 1],
            )
        return dst

    def project_v(wname, bv_bc, srcT):
        # V [128(lk), NT, H*65]: V[:, kt, h*65+v] = (srcT_chunk.T @ wv)[lk, h*64+v] + bv
        # col 65*h+64 is a ones column.
        V = m.tile([P, NT, H * 65], BF16, tag="m")
        w = load_w(wname)
        for kt in range(NT):
            ps = psum_pj.tile([P, NLC, LC], FP32, tag="pj")
            for dt in range(DT):
                lhsT = srcT[:, dt, kt * P:(kt + 1) * P]
                for lc in range(NLC):
                    nc.tensor.matmul(
                        ps[:, lc, :],
                        lhsT,
                        w[:, dt, lc * LC:(lc + 1) * LC],
                        start=(dt == 0),
                        stop=(dt == DT - 1),
                    )
            Vv = V[:, kt, :].rearrange("p (h c) -> p h c", c=65)
            for lc in range(NLC):
                nc.vector.tensor_add(
                    Vv[:, lc * 8:(lc + 1) * 8, 0:64],
                    ps[:, lc, :].rearrange("p (h c) -> p h c", c=64),
                    bv_bc[:, lc * LC:(lc + 1) * LC].rearrange(
                        "p (h c) -> p h c", c=64
                    ),
                )
        nc.vector.memset(
            V.rearrange("p a (h c) -> p a h c", c=65)[:, :, :, 64:65], 1.0
        )
        return V

    def attention(out_sa, causal, qt, kt, V, res_base=None, psums=None):
        for pr in range(NP):
            eS0 = m.tile([P, NT, L], BF16, tag="m")
            eS1 = m.tile([P, NT, L], BF16, tag="m")
            eS = [eS0, eS1]
            for lc in range(NLC):
                kts = list(range(4)) if (causal and lc == 0) else list(range(NT))
                for g0 in range(0, len(kts), 2):
                    grp = kts[g0:g0 + 2]
                    ps0 = psum_sc.tile([P, 2, LC], FP32, tag="sc")
                    ps1 = psum_sc.tile([P, 2, LC], FP32, tag="sc")
                    pss = [ps0, ps1]
                    for j, kt_ in enumerate(grp):
                        for i in range(2):
                            r0 = i * 64
                            nc.tensor.matmul(
                                pss[i][:, j, :],
                                kt[r0:r0 + 64, pr, kt_ * P:(kt_ + 1) * P],
                                qt[r0:r0 + 64, pr, lc * LC:(lc + 1) * LC],
                                start=True,
                                stop=True,
                                tile_position=(r0, 0),
                            )
                    for i in range(2):
                        nc.scalar.activation(
                            eS[i][:, grp[0]:grp[0] + len(grp),
                                  lc * LC:(lc + 1) * LC],
                            pss[i][:, 0:len(grp), :],
                            AF.Exp,
                            scale=0.125,
                        )
                    if causal:
                        for kt_ in grp:
                            if kt_ >= 4 * lc:
                                mi = kt_ - 4 * lc
                                for i in range(2):
                                    nc.vector.tensor_mul(
                                        eS[i][:, kt_, lc * LC:(lc + 1) * LC],
                                        eS[i][:, kt_, lc * LC:(lc + 1) * LC],
                                        mask_bf[:, mi, :],
                                    )
            # O^T rows 0:64 + softmax denominator row 64 (ones column of V)
            for i in range(2):
                h = 2 * pr + i
                av = psum_pj.tile([P, NLC, LC], FP32, tag="pj")
                for lc in range(NLC):
                    kts = list(range(4)) if (causal and lc == 0) else list(range(NT))
                    for j, kt_ in enumerate(kts):
                        nc.tensor.matmul(
                            av[0:65, lc, :],
                            V[:, kt_, h * 65:h * 65 + 65],
                            eS[i][:, kt_, lc * LC:(lc + 1) * LC],
                            start=(j == 0),
                            stop=(j == len(kts) - 1),
                        )
                ot = heads.tile([80, L], BF16, tag="ot")
                nc.vector.memset(ot[64:80, :], 0.0)
                otv = ot[0:65, :].rearrange("p (a b) -> p a b", a=NLC)
                nc.vector.tensor_copy(otv, av[0:65])
                otr = heads.tile([P, NT, 80], BF16, tag="otr")
                nc.sync.dma_start(otr, ot, transpose=True)
                rcp = heads.tile([P, NT, 1], FP32, tag="rcp")
                nc.vector.reciprocal(rcp, otr[:, :, 64:65])
                osv = out_sa.rearrange("p lt (hh c) -> p lt hh c", c=64)
                if res_base is None:
                    nc.vector.tensor_mul(
                        osv[:, :, h, :],
                        otr[:, :, 0:64],
                        rcp.broadcast_to([P, NT, 64]),
                    )
                else:
                    # fused: out = otr/denom + residual_base, with per-op
                    # free-dim sums collected for the next LayerNorm's mean
                    rbv = res_base.rearrange("p lt (hh c) -> p lt hh c", c=64)
                    for lt in range(NT):
                        nc.vector.scalar_tensor_tensor(
                            osv[:, lt, h, :],
                            otr[:, lt, 0:64],
                            rcp[:, lt, :],
                            rbv[:, lt, h, :],
                            op0=ALU.mult,
                            op1=ALU.add,
                            accum_out=psums[:, lt * H + h:lt * H + h + 1],
                        )

    def ln_stats_block(res_lt, sums_lt, ssq_lt):
        # Sigma r^2 on ScalarE (runs parallel to the DVE chain)
        dump2 = stage.tile([P, D], FP32, tag="zf")
        nc.scalar.activation(dump2, res_lt, AF.Square, accum_out=ssq_lt)

    def ln_block_scalars(sums_lt, ssq_lt, rsq_lt, mrs_lt):
        # mean = sums/D; var = ssq/D - mean^2; rstd = 1/sqrt(var+eps)
        mh = stage.tile([P, 1], FP32, tag="mh1")
        nc.vector.tensor_scalar_mul(mh, sums_lt, 1.0 / D)
        m2 = stage.tile([P, 1], FP32, tag="m21")
        nc.vector.tensor_mul(m2, mh, mh)
        v1 = stage.tile([P, 1], FP32, tag="v11")
        nc.vector.scalar_tensor_tensor(
            v1, ssq_lt, 1.0 / D, m2, op0=ALU.mult, op1=ALU.subtract
        )
        sq = stage.tile([P, 1], FP32, tag="sq1")
        nc.scalar.activation(sq, v1, AF.Sqrt, bias=eps_t[:, 0:1])
        nc.vector.reciprocal(rsq_lt, sq)
        nc.vector.tensor_mul(mrs_lt, mh, rsq_lt)

    def ln_half_scalars(sums, ssq, rsq, mrs, sl):
        # mean = sums/D; var = ssq/D - mean^2; rstd = 1/sqrt(var+eps)
        mh = stage.tile([P, 4, 1], FP32, tag="mh")
        nc.vector.tensor_scalar_mul(mh, sums[:, sl, :], 1.0 / D)
        m2 = stage.tile([P, 4, 1], FP32, tag="m2")
        nc.vector.tensor_mul(m2, mh, mh)
        v1 = stage.tile([P, 4, 1], FP32, tag="v1")
        nc.vector.scalar_tensor_tensor(
            v1, ssq[:, sl, :], 1.0 / D, m2, op0=ALU.mult, op1=ALU.subtract
        )
        sq = stage.tile([P, 4, 1], FP32, tag="sq")
        nc.scalar.activation(sq, v1, AF.Sqrt, bias=eps_t[:, 0:1])
        nc.vector.reciprocal(rsq[:, sl, :], sq)
        nc.vector.tensor_mul(mrs[:, sl, :], mh, rsq[:, sl, :])

    def ln_phase2(res, psums, emit_block):
        # res already contains attn_out + residual (fused in attention);
        # psums holds per-(lt, h) 64-col sums -> reduce to per-token mean.
        sums = stage.tile([P, NT, 1], FP32, tag="sums")
        ssq = stage.tile([P, NT, 1], FP32, tag="ssq")
        rsq = stage.tile([P, NT, 1], FP32, tag="rsq")
        mrs = stage.tile([P, NT, 1], FP32, tag="mrs")
        for lt in range(NT):
            nc.vector.reduce_sum(
                sums[:, lt, :], psums[:, lt * H:(lt + 1) * H],
                axis=mybir.AxisListType.X,
            )
            ln_stats_block(res[:, lt, :], None, ssq[:, lt, :])
            ln_block_scalars(sums[:, lt, :], ssq[:, lt, :],
                             rsq[:, lt, :], mrs[:, lt, :])
            emit_block(lt, res, rsq, mrs)

    def ln_phase(a_big, b_big, g_t, b_t, emit_block, res_name="res"):
        # residual r = a+b with free-dim sum accumulated in the same DVE op;
        # stats batched per half (4 blocks) so downstream work starts early.
        res = m.tile([P, NT, D], BF16, tag="m")
        sums = stage.tile([P, NT, 1], FP32, tag="sums")
        ssq = stage.tile([P, NT, 1], FP32, tag="ssq")
        rsq = stage.tile([P, NT, 1], FP32, tag="rsq")
        mrs = stage.tile([P, NT, 1], FP32, tag="mrs")
        for hf in range(2):
            lts = range(hf * 4, hf * 4 + 4)
            for lt in lts:
                nc.vector.scalar_tensor_tensor(
                    res[:, lt, :], a_big[:, lt, :], 1.0, b_big[:, lt, :],
                    op0=ALU.mult, op1=ALU.add, accum_out=sums[:, lt, :],
                )
                ln_stats_block(res[:, lt, :], sums[:, lt, :], ssq[:, lt, :])
            sl = slice(hf * 4, hf * 4 + 4)
            ln_half_scalars(sums, ssq, rsq, mrs, sl)
            for lt in lts:
                emit_block(lt, res, rsq, mrs)
        return res

    def ln_finish(dst, res_lt, rsq_lt, mrs_lt, g_t, b_t, via=None):
        z = via if via is not None else dst
        nc.vector.tensor_scalar(
            z, res_lt, rsq_lt, mrs_lt, op0=ALU.mult, op1=ALU.subtract
        )
        nc.vector.tensor_mul(dst, z, g_t)
        nc.vector.tensor_add(dst, dst, b_t)

    def tap(src_big):
        for lt in range(NT):
            o = stage.tile([P, D], FP32, tag="zf")
            nc.vector.tensor_copy(o, src_big[:, lt, :])
            nc.sync.dma_start(out_d.ap()[lt * P:(lt + 1) * P, :], o)

    # wq at the head of the SWDGE queue, first head-pair's columns first:
    # the very first matmul group only needs w[:, :, 0:128]
    wq_s = m.tile([P, DT, D], BF16, tag="m")
    nc.gpsimd.dma_start(
        wq_s[:, :, 0:P], ap("wq_m").rearrange("(dt p) c -> p dt c", p=P)[:, :, 0:P]
    )
    nc.gpsimd.dma_start(
        wq_s[:, :, P:D], ap("wq_m").rearrange("(dt p) c -> p dt c", p=P)[:, :, P:D]
    )
    # consts needed by the self-attention projections
    bqk = {}
    t_bqk_m = const.tile([P, 2, NP], FP32, tag="bqk_m")
    nc.gpsimd.dma_start(t_bqk_m, ap("bqk_m").rearrange("j (pr p) -> p j pr", p=P))
    bqk["bqk_m"] = t_bqk_m
    bcast = {}
    t_bv_m = const.tile([P, D], BF16, tag="bc_bv_m")
    nc.gpsimd.dma_start(t_bv_m, _bcast_ap(ap("bv_m")))
    bcast["bv_m"] = t_bv_m

    # ================= self attention =================
    def _xt_h1():
        nc.sync.dma_start(
            xT[:, :, LC:L], ap("x")[LC:L, :], transpose=True,
        )

    qt_s = project_qk("wq_m", bqk["bqk_m"], 0, xT, w_pre=wq_s,
                      lc_outer=True, between_lc=_xt_h1)
    kt_s = project_qk("wk_m", bqk["bqk_m"], 1, xT)
    nc.gpsimd.dma_start(mask_bf, ap("mask").rearrange("i p c -> p i c"))
    encT = m.tile([P, DT, L], BF16, tag="m")
    nc.sync.dma_start(encT, ap("enc"), transpose=True)
    V_s = project_v("wv_m", bcast["bv_m"], xT)
    # xT's slot is recycled after V_s projection (last reader)

    x_res = m.tile([P, NT, D], BF16, tag="m")
    nc.gpsimd.dma_start(x_res, ap("x").rearrange("(lt p) d -> p lt d", p=P))
    sa = m.tile([P, NT, D], BF16, tag="m")
    psums1 = stage.tile([P, NT * H], FP32, tag="pp")
    attention(sa, True, qt_s, kt_s, V_s, res_base=x_res, psums=psums1)
    if stop_after == "sa":
        tap(sa)
        return

    # remaining consts (first used at/after the cross projections)
    t_bqk_c = const.tile([P, 2, NP], FP32, tag="bqk_c")
    nc.gpsimd.dma_start(t_bqk_c, ap("bqk_c").rearrange("j (pr p) -> p j pr", p=P))
    bqk["bqk_c"] = t_bqk_c
    b1_col = const.tile([P, DFF // P], FP32)
    nc.gpsimd.dma_start(b1_col, ap("b1").rearrange("(ft p) -> p ft", p=P))
    for name in ("bv_c", "b2", "g1", "bb1", "g2", "bb2"):
        t = const.tile([P, D], BF16, tag=f"bc_{name}")
        nc.gpsimd.dma_start(t, _bcast_ap(ap(name)))
        bcast[name] = t

    # cross K/V projections (can fill PE gaps at the tail of self-attn)
    kt_c = project_qk("wk_c", bqk["bqk_c"], 1, encT)
    V_c = project_v("wv_c", bcast["bv_c"], encT)

    # ---- residual + LN1 -> x1 (bf16) and x1T ----
    x1 = m.tile([P, NT, D], BF16, tag="m")
    x1T = m.tile([P, DT, L], BF16, tag="m")

    def emit_ln1(lt, res, rsq, mrs):
        z = stage.tile([P, D], BF16, tag="zb")
        ln_finish(x1[:, lt, :], res[:, lt, :], rsq[:, lt, :], mrs[:, lt, :],
                  bcast["g1"], bcast["bb1"], via=z)
        nc.sync.dma_start(
            x1T[:, :, lt * P:(lt + 1) * P], x1[:, lt, :], transpose=True
        )

    ln_phase2(sa, psums1, emit_ln1)
    if stop_after == "x1":
        tap(x1)
        return

    # ================= cross attention =================
    qt_c = project_qk("wq_c", bqk["bqk_c"], 0, x1T, lc_outer=True)
    ca = m.tile([P, NT, D], BF16, tag="m")
    psums2 = stage.tile([P, NT * H], FP32, tag="pp")
    attention(ca, False, qt_c, kt_c, V_c, res_base=x1, psums=psums2)
    if stop_after == "ca":
        tap(ca)
        return

    # prefetch FFN quarter-0 weights so FFN1 starts inside the LN2 window
    w1_pre = m.tile([P, DT, FT * P], BF16, tag="m")
    nc.gpsimd.dma_start(
        w1_pre, ap("w1")[:, 0:FT * P].rearrange("(dt p) c -> p dt c", p=P),
    )
    w2_pre = m.tile([P, FT, D], BF16, tag="m")
    nc.gpsimd.dma_start(
        w2_pre, ap("w2")[0:FT * P, :].rearrange("(ft p) c -> p ft c", p=P),
    )

    # ---- residual + LN2 -> x2 (bf16) and x2T ----
    x2 = m.tile([P, NT, D], BF16, tag="m")
    x2T = m.tile([P, DT, L], BF16, tag="m")

    def emit_ln2(lt, res, rsq, mrs):
        z = stage.tile([P, D], BF16, tag="zb")
        ln_finish(x2[:, lt, :], res[:, lt, :], rsq[:, lt, :], mrs[:, lt, :],
                  bcast["g2"], bcast["bb2"], via=z)
        nc.sync.dma_start(
            x2T[:, :, lt * P:(lt + 1) * P], x2[:, lt, :], transpose=True
        )

    ln_phase2(ca, psums2, emit_ln2)
    if stop_after == "x2":
        tap(x2)
        return

    # ================= FFN (dff quarters) =================
    y_bf = m.tile([P, NT, D], BF16, tag="m")
    res3 = None
    rsq3 = stage.tile([P, NT, 1], FP32, tag="rsq")
    mrs3 = stage.tile([P, NT, 1], FP32, tag="mrs")

    def final_block(lt):
        ln_block_scalars(sums3[:, lt, :], ssq3[:, lt, :],
                         rsq3[:, lt, :], mrs3[:, lt, :])
        if True:
            # z/g-mul in bf16 (2x/4x DVE modes); the final +b TT widens to
            # fp32 so the store stays on the HWDGE ring (no Q7 desc-gen)
            o = stage.tile([P, D], FP32, tag="zf")
            z = stage.tile([P, D], BF16, tag="zb")
            nc.vector.tensor_scalar(
                z, res3[:, lt, :], rsq3[:, lt, :], mrs3[:, lt, :],
                op0=ALU.mult, op1=ALU.subtract,
            )
            t2 = stage.tile([P, D], BF16, tag="zb")
            nc.vector.tensor_mul(t2, z, bcast["g2"])
            nc.vector.tensor_add(o, t2, bcast["bb2"])
            nc.sync.dma_start(out_d.ap()[lt * P:(lt + 1) * P, :], o)
    sums3 = stage.tile([P, NT, 1], FP32, tag="sums")
    ssq3 = stage.tile([P, NT, 1], FP32, tag="ssq")
    for q in range(FQ):
        if q == 0:
            w1 = w1_pre
        else:
            w1 = m.tile([P, DT, FT * P], BF16, tag="m")
            nc.gpsimd.dma_start(
                w1,
                ap("w1")[:, q * FT * P:(q + 1) * FT * P].rearrange(
                    "(dt p) c -> p dt c", p=P
                ),
            )
        h1 = m.tile([P, FT, L], BF16, tag="m")
        if q == 0:
            # lc-outer: h1 for the first l-half only needs x2T's first 512
            # l-columns (LN2 blocks 0..3) -> FFN starts during LN2.
            for lc in range(NLC):
                for ft in range(FT):
                    ps = psum_sc.tile([P, 1, LC], FP32, tag="sc")
                    for dt in range(DT):
                        nc.tensor.matmul(
                            ps[:, 0, :],
                            w1[:, dt, ft * P:(ft + 1) * P],
                            x2T[:, dt, lc * LC:(lc + 1) * LC],
                            start=(dt == 0),
                            stop=(dt == DT - 1),
                        )
                    nc.scalar.activation(
                        h1[:, ft, lc * LC:(lc + 1) * LC],
                        ps[:, 0, :],
                        AF.Relu,
                        bias=b1_col[:, q * FT + ft:q * FT + ft + 1],
                    )
        else:
            for ft in range(FT):
                ps = psum_sc.tile([P, NLC, LC], FP32, tag="sc")
                for dt in range(DT):
                    lhsT = w1[:, dt, ft * P:(ft + 1) * P]
                    for lc in range(NLC):
                        nc.tensor.matmul(
                            ps[:, lc, :],
                            lhsT,
                            x2T[:, dt, lc * LC:(lc + 1) * LC],
                            start=(dt == 0),
                            stop=(dt == DT - 1),
                        )
                nc.scalar.activation(
                    h1[:, ft, :].rearrange("p (a b) -> p a b", a=NLC),
                    ps,
                    AF.Relu,
                    bias=b1_col[:, q * FT + ft:q * FT + ft + 1],
                )
        if q == 0:
            w2 = w2_pre
        else:
            w2 = m.tile([P, FT, D], BF16, tag="m")
            nc.gpsimd.dma_start(
                w2,
                ap("w2")[q * FT * P:(q + 1) * FT * P, :].rearrange(
                    "(ft p) c -> p ft c", p=P
                ),
            )
        if q == FQ - 1:
            res3 = m.tile([P, NT, D], BF16, tag="m")
            # pre-residual x2 + y(q0..q2): runs on DVE during FFN1 of the
            # last quarter, so the q3 evacuation is a single op per block
            pre3 = m.tile([P, NT, D], BF16, tag="m")
            for lb in range(NT):
                nc.vector.tensor_add(pre3[:, lb, :], y_bf[:, lb, :], x2[:, lb, :])
        for lb in range(NT):
            ps = psum_pj.tile([P, NLC, LC], FP32, tag="pj")
            for ft in range(FT):
                lhsT = h1[:, ft, lb * P:(lb + 1) * P]
                for lc in range(NLC):
                    nc.tensor.matmul(
                        ps[:, lc, :],
                        lhsT,
                        w2[:, ft, lc * LC:(lc + 1) * LC],
                        start=(ft == 0),
                        stop=(ft == FT - 1),
                    )
            psv = ps.rearrange("p a b -> p (a b)")
            if q == 0:
                nc.vector.tensor_add(y_bf[:, lb, :], psv, bcast["b2"])
            elif q < FQ - 1:
                nc.vector.tensor_add(y_bf[:, lb, :], y_bf[:, lb, :], psv)
            else:
                nc.vector.scalar_tensor_tensor(
                    res3[:, lb, :], psv, 1.0, pre3[:, lb, :],
                    op0=ALU.mult, op1=ALU.add, accum_out=sums3[:, lb, :],
                )
                dump2 = stage.tile([P, D], FP32, tag="zf")
                nc.scalar.activation(
                    dump2, res3[:, lb, :], AF.Square, accum_out=ssq3[:, lb, :]
                )
                final_block(lb)




_NC_CACHE = {}


def _get_nc(stop_after=None):
    key = stop_after
    if key not in _NC_CACHE:
        _NC_CACHE[key] = build(stop_after)
    return _NC_CACHE[key]


def _pack_weights(inputs):
    """Host-side prepack: cast to bf16 and lay out as the kernel expects."""
    f32 = lambda k: np.ascontiguousarray(np.asarray(inputs[k], dtype=np.float32))
    bf = lambda a: np.ascontiguousarray(np.asarray(a, dtype=NP_BF16))

    def attn_w(k):
        # [H, D, DK] -> [D, H*DK] bf16
        w = f32(k).transpose(1, 0, 2).reshape(D, H * DK)
        return bf(w)

    return {
        "wq_m": attn_w("m_wq"), "wk_m": attn_w("m_wk"), "wv_m": attn_w("m_wv"),
        "wq_c": attn_w("c_wq"), "wk_c": attn_w("c_wk"), "wv_c": attn_w("c_wv"),
        "bqk_m": np.ascontiguousarray(
            np.stack([f32("m_bq").reshape(-1), f32("m_bk").reshape(-1)])
        ),
        "bqk_c": np.ascontiguousarray(
            np.stack([f32("c_bq").reshape(-1), f32("c_bk").reshape(-1)])
        ),
        "bv_m": bf(f32("m_bv").reshape(-1)),
        "bv_c": bf(f32("c_bv").reshape(-1)),
        "w1": bf(f32("ff_w1")),
        "w2": bf(f32("ff_w2")),
        "b1": f32("ff_b1"),
        "b2": bf(f32("ff_b2")),
        "g1": bf(f32("ln1_g")), "bb1": bf(f32("ln1_b")),
        "g2": bf(f32("ln2_g")), "bb2": bf(f32("ln2_b")),
    }


_MASK = None


def _causal_mask():
    global _MASK
    if _MASK is None:
        kk = np.arange(P)[None, :, None]
        qq = np.arange(LC)[None, None, :]
        ii = np.arange(4)[:, None, None]
        _MASK = np.ascontiguousarray(
            (qq >= kk + ii * P).astype(NP_BF16)
        )
    return _MASK


def _make_in_maps(inputs):
    xs = np.ascontiguousarray(
        np.asarray(inputs["decoder_embedding"], dtype=np.float32).astype(NP_BF16)
    )
    es = np.ascontiguousarray(
        np.asarray(inputs["encoder_output"], dtype=np.float32).astype(NP_BF16)
    )
    packed = _pack_weights(inputs)
    packed["mask"] = _causal_mask()
    return [{**packed, "x": xs[b], "enc": es[b]} for b in range(B)]


def _gather(res):
    return np.stack([res.results[b]["out"] for b in range(B)], axis=0).astype(np.float32)


def kernel(**inputs):
    nc = _get_nc()
    res = run_bass_kernel_spmd(nc, _make_in_maps(inputs), core_ids=list(range(B)))
    return _gather(res)
